# revision 27
# baseline (speedup 1.0000x reference)
"""KGAttentionLayer Trainium2 kernel (v3: row-packed scores, pair-batched
exp, DVE post-processing).

Sharding: 8 cores = (batch 2) x (query-block 4). Core c handles batch
b=c//4, query rows [j*512, (j+1)*512) of that batch (j=c%4). Each core
computes k/v/kg projections for its whole batch (duplicated within the
4-core batch group), attention for its 512 queries over all 16 heads,
and the gate/out-proj/residual for its rows.

Attention structure: heads are processed in PAIRS. The k tensor for a
pair is stacked on partitions (even head dims at 0:64, odd at 64:128)
so the two K=64 score matmuls run concurrently in different PE row
groups (tile_position row packing). Both score chunks land in one
2-bank PSUM tile [128, 1024] and a single ACT exp covers both, halving
ACT instruction count. The av matmuls accumulate a pair tile [65, 1024]
(numerator rows 0:64, denominator row 64 via the ones column in vslab).

Layouts (host pre-transposes; the PE never transposes on device):
  xT      [1024, 2048]  x[b].T, rolled so the core's query block is at
                        columns [0, 512)
  qts2    [128, 8, 512] pair-stacked q (even head at parts 0:64)
  kta2    [128, 2, M]   per-block pair-stacked k
  vslab   [128, 18, 4*65] per-head 64 v-cols (+bias baked in) plus a
          ones column -> the attn@v matmul emits the softmax numerator
          in PSUM rows 0..63 and the denominator in row 64.
"""

import sys

sys.path.insert(0, "/opt/trn_rl_repo")

import numpy as np

import concourse.bass as bass
import concourse.mybir as mybir
import concourse.tile as tile
from concourse import bacc
from concourse.bass_utils import run_bass_kernel_spmd

F32 = mybir.dt.float32
BF16 = mybir.dt.bfloat16
FP8 = mybir.dt.float8e4
DR = mybir.MatmulPerfMode.DoubleRow
AF = mybir.ActivationFunctionType
OP = mybir.AluOpType

# Weights are pre-scaled by 16 on the host so all e4m3 weight values are
# normal (lim=1/32 puts half of them in the subnormal range otherwise);
# the 1/16 is folded into the post-matmul bias/scale stage. For v the 16
# is left in and cancelled by a 16.0 (instead of 1.0) ones-column in
# vslab: numerator and denominator both carry 16x, the softmax ratio is
# exact.
WSCALE = 16.0
RSCALE = 1.0 / WSCALE

D = 1024
H = 16
HD = 64
B = 2
L = 2048
E = 256
LBLK = 512          # queries per core
M = L + E           # 2304 attended positions
NMC = M // 128      # 18 m-chunks
VW = 68             # vslab per-head slot (65 used; padded so the mc
                    # stride 4*VW=272 B is 16-aligned for DoubleRow LDW)
N_CORES = 8

_CACHE = {}


def _build(repeat=1, do_proj=True, do_attn=True, do_out=True):
    nc = bacc.Bacc("TRN2", target_bir_lowering=False, debug=False,
                   num_devices=N_CORES)

    dram = {}

    def din(name, shape, dt=BF16):
        dram[name] = nc.dram_tensor(name, shape, dt, kind="ExternalInput")
        return dram[name]

    xT = din("xT", [D, L], FP8)
    xR = din("xR", [128, 8, LBLK], BF16)   # residual slice of x
    kgT = din("kgT", [D, E], FP8)
    WqT = din("WqT", [D, D], FP8)
    WkT = din("WkT", [D, D], FP8)
    WkkT = din("WkkT", [D, D], FP8)
    WvT = din("WvT", [D, D], FP8)
    WkvT = din("WkvT", [D, D], FP8)
    WoT = din("WoT", [D, D], FP8)
    WgT = din("WgT", [D, D], FP8)
    bq = din("bq", [128, 8], F32)      # col g = (bias*0.125)[g*128:(g+1)*128]
    bk = din("bk", [128, 8], F32)
    bkk = din("bkk", [128, 8], F32)
    bo = din("bo", [128, 8], F32)
    bge = din("bge", [128, 8], F32)    # bg + Wg[:,1024:] @ kg_mean(batch)
    bvb = din("bvb", [128, D], F32)    # np.tile(bv, (128,1))
    bkvb = din("bkvb", [128, D], F32)

    OUTT = nc.dram_tensor("OUTT", [D, LBLK], F32, kind="ExternalOutput")

    def w8(pool, W, g, tag="w8"):
        t = pool.tile([128, 8, 128], FP8, tag=tag, name=f"{tag}_{W.name}_{g}")
        nc.sync.dma_start(
            t[:], W.ap().rearrange("(kk p) d -> p kk d", p=128)
            [:, :, g * 128:(g + 1) * 128])
        return t

    from contextlib import ExitStack

    with tile.TileContext(nc) as tc, ExitStack() as ctx:
        persist = ctx.enter_context(tc.tile_pool(name="persist", bufs=1))
        wpool = ctx.enter_context(tc.tile_pool(name="wpool", bufs=2))
        spool = ctx.enter_context(tc.tile_pool(name="spool", bufs=2))
        epool = ctx.enter_context(tc.tile_pool(name="epool", bufs=3))
        blkpool = ctx.enter_context(tc.tile_pool(name="blkpool", bufs=2))
        # PSUM: scores 2x[128,1024] (4 banks) + av 1x[65,1024] (2 banks)
        #       + proj 2x[128,512] (2 banks) = 8 banks
        psS = ctx.enter_context(tc.tile_pool(name="psS", bufs=2, space="PSUM"))
        psAV = ctx.enter_context(tc.tile_pool(name="psAV", bufs=1, space="PSUM"))
        psP = ctx.enter_context(tc.tile_pool(name="psP", bufs=2, space="PSUM"))

        # ---- resident loads ----
        xts = persist.tile([128, 8, L], FP8, tag="xts")
        nc.sync.dma_start(xts[:], xT.ap().rearrange("(kk p) l -> p kk l", p=128))
        xrs = persist.tile([128, 8, LBLK], BF16, tag="xrs")
        nc.sync.dma_start(xrs[:], xR.ap())
        kgts = persist.tile([128, 8, E], FP8, tag="kgts")
        nc.sync.dma_start(kgts[:], kgT.ap().rearrange("(kk p) e -> p kk e", p=128))
        biases = {}
        for nm in ("bq", "bk", "bkk", "bo", "bge"):
            t = persist.tile([128, 8], F32, tag=nm, name=nm + "_sb")
            nc.sync.dma_start(t[:], dram[nm].ap())
            biases[nm] = t
        bvbs = persist.tile([128, D], F32, tag="bvbs")
        nc.sync.dma_start(bvbs[:], bvb.ap())
        bkvbs = persist.tile([128, D], F32, tag="bkvbs")
        nc.sync.dma_start(bkvbs[:], bkvb.ap())
        onesv = persist.tile([128, NMC, 16, 1], FP8, tag="onesv")
        nc.vector.memset(onesv[:], WSCALE)
        ones1 = persist.tile([1, 64], BF16, tag="ones1")
        nc.vector.memset(ones1[:], 1.0)

        qts2 = persist.tile([128, 8, LBLK], BF16, tag="qts2")
        outTs = persist.tile([128, 8, LBLK], FP8, tag="outTs")
        # persistent vslab: all 16 heads (head h slot at h*VW), m-chunk major
        vslab = persist.tile([128, NMC, 16 * VW], FP8, tag="vslab")
        vstage = persist.tile([128, 16, D], FP8, tag="vstage")
        klocal = persist.tile([128, 8, LBLK], FP8, tag="klocal")
        vlocal = persist.tile([128, 4, D], FP8, tag="vlocal")
        dramp = ctx.enter_context(tc.tile_pool(name="dramp", bufs=1,
                                               space="DRAM"))

        # repeat>1 builds a NEFF that runs the whole body `repeat` times --
        # used only by the timing harness (slope removes the RPC overhead).
        for _rep in range(repeat):
            # ---- ones columns for the whole vslab ----
            nc.vector.tensor_copy(
                vslab[:].rearrange("p mc (h c) -> p mc h c", c=VW)
                [:, :, :, 64:65], onesv[:])

            # ---- local k projection (own 512 columns only) ----
            for g in range(8 if do_proj else 0):
                wk = w8(wpool, WkT, g)
                ps = psP.tile([128, LBLK], F32, tag="psP", name=f"klp{g}")
                for t in range(4):
                    nc.tensor.matmul(ps[:], wk[:, 2 * t:2 * t + 2, :],
                                     xts[:, 2 * t:2 * t + 2, 0:LBLK],
                                     start=(t == 0), stop=(t == 3),
                                     perf_mode=DR)
                nc.vector.tensor_scalar(
                    klocal[:, g, :], ps[:], RSCALE,
                    biases["bk"][:, g:g + 1], OP.mult, OP.add)

            # ---- local v projection (own 512 rows, all 1024 v-dims) ----
            if do_proj:
                wvf = wpool.tile([128, 8, D], FP8, tag="wvf", name="wvf")
                nc.sync.dma_start(
                    wvf[:], WvT.ap().rearrange("(kk p) d -> p kk d", p=128))
                for mcl in range(4):
                    for half in range(2):
                        ps = psP.tile([128, 512], F32, tag="psP",
                                      name=f"vl{mcl}_{half}")
                        for t in range(4):
                            nc.tensor.matmul(
                                ps[:],
                                xts[:, 2 * t:2 * t + 2,
                                    mcl * 128:(mcl + 1) * 128],
                                wvf[:, 2 * t:2 * t + 2,
                                    half * 512:half * 512 + 512],
                                start=(t == 0), stop=(t == 3), perf_mode=DR)
                        nc.vector.tensor_add(
                            vlocal[:, mcl, half * 512:half * 512 + 512],
                            ps[:], bvbs[:, half * 512:half * 512 + 512])

                # ---- AllGather k+v within the 4-core batch group ----
                agin = dramp.tile([2, 128, 8, LBLK], FP8, tag="agin")
                agout = dramp.tile([4, 2, 128, 8, LBLK], FP8, tag="agout")
                nc.sync.dma_start(agin[0], klocal[:])
                nc.sync.dma_start(
                    agin[1].rearrange("p g m -> p (g m)"),
                    vlocal[:].rearrange("p a b -> p (a b)"))
                nc.gpsimd.collective_compute(
                    "AllGather", mybir.AluOpType.bypass,
                    replica_groups=[[0, 1, 2, 3], [4, 5, 6, 7]],
                    ins=[agin.opt()], outs=[agout.opt()])
                # v: stage then scatter into vslab head slots
                nc.sync.dma_start(
                    vstage[:].rearrange("p (jj x) b -> p jj (x b)", jj=4),
                    agout[:, 1].rearrange("jj p g m -> p jj (g m)"))
                for mc in range(16):
                    nc.vector.tensor_copy(
                        vslab[:, mc, :].rearrange("p (h c) -> p h c", c=VW)
                        [:, :, 0:64],
                        vstage[:, mc, :].rearrange("p (h c) -> p h c", c=64))

                # ---- kg v projection (replicated, all heads) ----
                wkvf = wpool.tile([128, 8, D], FP8, tag="wvf", name="wkvf")
                nc.sync.dma_start(
                    wkvf[:], WkvT.ap().rearrange("(kk p) d -> p kk d", p=128))
                for mcl in range(2):
                    for half in range(2):
                        ps = psP.tile([128, 512], F32, tag="psP",
                                      name=f"vkg{mcl}_{half}")
                        for t in range(4):
                            nc.tensor.matmul(
                                ps[:],
                                kgts[:, 2 * t:2 * t + 2,
                                     mcl * 128:(mcl + 1) * 128],
                                wkvf[:, 2 * t:2 * t + 2,
                                     half * 512:half * 512 + 512],
                                start=(t == 0), stop=(t == 3), perf_mode=DR)
                        nc.vector.tensor_add(
                            vslab[:, 16 + mcl, :]
                            .rearrange("p (h c) -> p h c", c=VW)
                            [:, 8 * half:8 * half + 8, 0:64],
                            ps[:].rearrange("p (h c) -> p h c", c=64),
                            bkvbs[:, half * 512:half * 512 + 512]
                            .rearrange("p (h c) -> p h c", c=64))

            # ---- q projection (all pairs) ----
            for g in range(8 if do_proj else 0):
                wq = w8(wpool, WqT, g)
                ps = psP.tile([128, LBLK], F32, tag="psP", name=f"qp{g}")
                for t in range(4):
                    nc.tensor.matmul(ps[:], wq[:, 2 * t:2 * t + 2, :],
                                     xts[:, 2 * t:2 * t + 2, 0:LBLK],
                                     start=(t == 0), stop=(t == 3),
                                     perf_mode=DR)
                nc.vector.tensor_scalar(
                    qts2[:, g, :], ps[:], 0.125 * RSCALE,
                    biases["bq"][:, g:g + 1], OP.mult, OP.add)

            for blk in range(4):
                g0 = 2 * blk
                # kta2: pair-stacked k for this block's 2 pairs (fp8)
                kta2 = blkpool.tile([128, 2, M], FP8, tag="kta2")
                if do_proj:
                    # gathered k blocks
                    for jj in range(4):
                        nc.sync.dma_start(
                            kta2[:, :, jj * LBLK:(jj + 1) * LBLK],
                            agout[jj, 0, :, g0:g0 + 2, :])
                    # kg k projection for this block's pairs
                    for gi, g in enumerate((g0, g0 + 1)):
                        wkk = w8(wpool, WkkT, g)
                        ps = psP.tile([128, E], F32, tag="psP",
                                      name=f"kkp{g}")
                        for t in range(4):
                            nc.tensor.matmul(ps[:], wkk[:, 2 * t:2 * t + 2, :],
                                             kgts[:, 2 * t:2 * t + 2, :],
                                             start=(t == 0), stop=(t == 3),
                                             perf_mode=DR)
                        nc.vector.tensor_scalar(
                            kta2[:, gi, L:M], ps[:], RSCALE,
                            biases["bkk"][:, g:g + 1], OP.mult, OP.add)

                # ---- attention for pairs g0, g0+1 ----
                for pr in range(2 if do_attn else 0):
                    g = g0 + pr            # pair index == qts2 column
                    h0 = 4 * blk + 2 * pr  # global even head
                    avp = psAV.tile([65, 1024], F32, tag="psAV",
                                    name=f"av{g}")
                    for mc in range(NMC):
                        sp = psS.tile([128, 1024], F32, tag="psS",
                                      name=f"sp{g}_{mc}")
                        nc.tensor.matmul(sp[:, 0:512],
                                         kta2[0:64, pr,
                                              mc * 128:(mc + 1) * 128],
                                         qts2[0:64, g, :],
                                         start=True, stop=True)
                        nc.tensor.matmul(sp[:, 512:1024],
                                         kta2[64:128, pr,
                                              mc * 128:(mc + 1) * 128],
                                         qts2[64:128, g, :],
                                         start=True, stop=True)
                        if mc % 2 == 0:
                            et2 = epool.tile([128, 2, 1024], FP8, tag="et",
                                             name=f"et{g}_{mc}")
                        nc.scalar.activation(et2[:, mc % 2, :], sp[:], AF.Exp)
                        if mc % 2 == 1:
                            nc.tensor.matmul(
                                avp[:, 0:512],
                                vslab[:, mc - 1:mc + 1,
                                      h0 * VW:h0 * VW + 65],
                                et2[:, :, 0:512],
                                start=(mc == 1), stop=(mc == NMC - 1),
                                perf_mode=DR)
                            nc.tensor.matmul(
                                avp[:, 512:1024],
                                vslab[:, mc - 1:mc + 1,
                                      (h0 + 1) * VW:(h0 + 1) * VW + 65],
                                et2[:, :, 512:1024],
                                start=(mc == 1), stop=(mc == NMC - 1),
                                perf_mode=DR)
                    # post-process (reciprocal must read from SBUF, not PSUM)
                    den = spool.tile([1, 1024], F32, tag="den",
                                     name=f"den{g}")
                    nc.vector.tensor_copy(den[:], avp[64:65, :])
                    rec = spool.tile([1, 1024], F32, tag="rec",
                                     name=f"rec{g}")
                    nc.vector.reciprocal_approx_fast(rec[:], den[:])
                    recr = spool.tile([1, 1024], BF16, tag="recr",
                                      name=f"recr{g}")
                    nc.vector.tensor_copy(recr[:], rec[:])
                    avs = spool.tile([64, 1024], F32, tag="avs",
                                     name=f"avs{g}")
                    nc.vector.tensor_copy(avs[:], avp[0:64, :])
                    rpE = psP.tile([64, 512], F32, tag="psP", name=f"rpE{g}")
                    nc.tensor.matmul(rpE[:], ones1[:], recr[:, 0:512],
                                     start=True, stop=True)
                    rpO = psP.tile([64, 512], F32, tag="psP", name=f"rpO{g}")
                    nc.tensor.matmul(rpO[:], ones1[:], recr[:, 512:1024],
                                     start=True, stop=True)
                    nc.vector.tensor_mul(outTs[0:64, g, :], avs[:, 0:512],
                                         rpE[:])
                    nc.vector.tensor_mul(outTs[64:128, g, :],
                                         avs[:, 512:1024], rpO[:])

            # ---- out-proj + gate + residual ----
            for g in range(8 if do_out else 0):
                wo = w8(wpool, WoT, g)
                wg = w8(wpool, WgT, g)
                pp = psP.tile([128, LBLK], F32, tag="psP", name=f"pp{g}")
                for t in range(4):
                    nc.tensor.matmul(pp[:], wo[:, 2 * t:2 * t + 2, :],
                                     outTs[:, 2 * t:2 * t + 2, :],
                                     start=(t == 0), stop=(t == 3),
                                     perf_mode=DR)
                pj = spool.tile([128, LBLK], F32, tag="pj", name=f"pj{g}")
                nc.vector.tensor_scalar(pj[:], pp[:], RSCALE,
                                        biases["bo"][:, g:g + 1],
                                        OP.mult, OP.add)
                gp = psP.tile([128, LBLK], F32, tag="psP", name=f"gp{g}")
                for t in range(4):
                    nc.tensor.matmul(gp[:], wg[:, 2 * t:2 * t + 2, :],
                                     outTs[:, 2 * t:2 * t + 2, :],
                                     start=(t == 0), stop=(t == 3),
                                     perf_mode=DR)
                gt = spool.tile([128, LBLK], F32, tag="gt", name=f"gt{g}")
                nc.scalar.activation(gt[:], gp[:], AF.Sigmoid,
                                     bias=biases["bge"][:, g:g + 1],
                                     scale=RSCALE)
                d1 = spool.tile([128, LBLK], F32, tag="fin", name=f"d1{g}")
                nc.vector.tensor_sub(d1[:], pj[:], xrs[:, g, :])
                d2 = spool.tile([128, LBLK], F32, tag="fin", name=f"d2{g}")
                nc.vector.tensor_mul(d2[:], d1[:], gt[:])
                fo = spool.tile([128, LBLK], F32, tag="fin", name=f"fo{g}")
                nc.vector.tensor_add(fo[:], d2[:], xrs[:, g, :])
                nc.sync.dma_start(OUTT.ap()[g * 128:(g + 1) * 128, :], fo[:])

    nc.compile()
    return nc


def kernel(x, kg_embeds, Wq, bq, Wk, bk, Wv, bv, Wkk, bkk, Wkv, bkv,
           Wo, bo, Wg, bg):
    import ml_dtypes
    bf16 = ml_dtypes.bfloat16
    f8 = ml_dtypes.float8_e4m3

    x = np.asarray(x, np.float32)
    kg_embeds = np.asarray(kg_embeds, np.float32)
    ws = {k: np.asarray(v, np.float32) for k, v in dict(
        Wq=Wq, bq=bq, Wk=Wk, bk=bk, Wv=Wv, bv=bv, Wkk=Wkk, bkk=bkk,
        Wkv=Wkv, bkv=bkv, Wo=Wo, bo=bo, Wg=Wg, bg=bg).items()}

    if "nc" not in _CACHE:
        _CACHE["nc"] = _build()
    nc = _CACHE["nc"]

    def col8(v):
        return np.ascontiguousarray(v.reshape(8, 128).T)

    def w16(w):
        # weights pre-scaled by WSCALE=16 before fp8 so they stay normal
        return np.ascontiguousarray((w.T * 16.0).astype(f8))

    # pair-stacked bias layouts: column g holds dims for head pair g
    # (even head dims at rows 0:64, odd head dims at rows 64:128) -- which
    # is exactly the contiguous [g*128:(g+1)*128] slice, same as col8.
    shared = {
        "WqT": w16(ws["Wq"]),
        "WkT": w16(ws["Wk"]),
        "WkkT": w16(ws["Wkk"]),
        "WvT": w16(ws["Wv"]),
        "WkvT": w16(ws["Wkv"]),
        "WoT": w16(ws["Wo"]),
        "WgT": w16(ws["Wg"][:, :D]),
        "bq": col8(ws["bq"] * 0.125),
        "bk": col8(ws["bk"]),
        "bkk": col8(ws["bkk"]),
        "bo": col8(ws["bo"]),
        # vslab holds 16*v (weights pre-scaled, bias scaled to match); the
        # 16 cancels against the 16.0 ones-column in the softmax ratio
        "bvb": np.ascontiguousarray(np.tile(ws["bv"] * 16.0, (128, 1))),
        "bkvb": np.ascontiguousarray(np.tile(ws["bkv"] * 16.0, (128, 1))),
    }

    in_maps = []
    for c in range(N_CORES):
        b, j = divmod(c, 4)
        # roll the core's query block to columns [0, 512); k/v attend over
        # all columns, so their (rolled) order is irrelevant to softmax
        xb = np.ascontiguousarray(np.roll(x[b].T, -j * LBLK, axis=1))
        kgm = kg_embeds[b].mean(axis=0)
        bge = ws["bg"] + ws["Wg"][:, D:] @ kgm
        m = dict(shared)
        m["xT"] = xb.astype(f8)
        m["xR"] = np.ascontiguousarray(
            xb[:, :LBLK].reshape(8, 128, LBLK).transpose(1, 0, 2)).astype(bf16)
        m["kgT"] = np.ascontiguousarray(kg_embeds[b].T.astype(f8))
        m["bge"] = col8(bge)
        in_maps.append(m)

    _CACHE["in_maps"] = in_maps
    res = run_bass_kernel_spmd(nc, in_maps, core_ids=list(range(N_CORES)))
    out = np.empty((B, L, D), np.float32)
    for c in range(N_CORES):
        b, j = divmod(c, 4)
        out[b, j * LBLK:(j + 1) * LBLK, :] = res.results[c]["OUTT"].T
    return out


# revision 29
# speedup vs baseline: 1.3942x; 1.3942x over previous
"""KGAttentionLayer Trainium2 kernel (v3: row-packed scores, pair-batched
exp, DVE post-processing).

Sharding: 8 cores = (batch 2) x (query-block 4). Core c handles batch
b=c//4, query rows [j*512, (j+1)*512) of that batch (j=c%4). Each core
computes k/v/kg projections for its whole batch (duplicated within the
4-core batch group), attention for its 512 queries over all 16 heads,
and the gate/out-proj/residual for its rows.

Attention structure: heads are processed in PAIRS. The k tensor for a
pair is stacked on partitions (even head dims at 0:64, odd at 64:128)
so the two K=64 score matmuls run concurrently in different PE row
groups (tile_position row packing). Both score chunks land in one
2-bank PSUM tile [128, 1024] and a single ACT exp covers both, halving
ACT instruction count. The av matmuls accumulate a pair tile [65, 1024]
(numerator rows 0:64, denominator row 64 via the ones column in vslab).

Layouts (host pre-transposes; the PE never transposes on device):
  xT      [1024, 2048]  x[b].T, rolled so the core's query block is at
                        columns [0, 512)
  qts2    [128, 8, 512] pair-stacked q (even head at parts 0:64)
  kta2    [128, 2, M]   per-block pair-stacked k
  vslab   [128, 18, 4*65] per-head 64 v-cols (+bias baked in) plus a
          ones column -> the attn@v matmul emits the softmax numerator
          in PSUM rows 0..63 and the denominator in row 64.
"""

import sys

sys.path.insert(0, "/opt/trn_rl_repo")

import numpy as np

import concourse.bass as bass
import concourse.mybir as mybir
import concourse.tile as tile
from concourse import bacc
from concourse.bass_utils import run_bass_kernel_spmd

F32 = mybir.dt.float32
BF16 = mybir.dt.bfloat16
FP8 = mybir.dt.float8e4
DR = mybir.MatmulPerfMode.DoubleRow
AF = mybir.ActivationFunctionType
OP = mybir.AluOpType

# Weights are pre-scaled by 16 on the host so all e4m3 weight values are
# normal (lim=1/32 puts half of them in the subnormal range otherwise);
# the 1/16 is folded into the post-matmul bias/scale stage. For v the 16
# is left in and cancelled by a 16.0 (instead of 1.0) ones-column in
# vslab: numerator and denominator both carry 16x, the softmax ratio is
# exact.
WSCALE = 16.0
RSCALE = 1.0 / WSCALE

D = 1024
H = 16
HD = 64
B = 2
L = 2048
E = 256
LBLK = 512          # queries per core
M = L + E           # 2304 attended positions
NMC = M // 128      # 18 m-chunks
VW = 68             # vslab per-head slot (65 used; padded so the mc
                    # stride 4*VW=272 B is 16-aligned for DoubleRow LDW)
N_CORES = 8

_CACHE = {}


def _build(repeat=1, do_proj=True, do_attn=True, do_out=True):
    nc = bacc.Bacc("TRN2", target_bir_lowering=False, debug=False,
                   num_devices=N_CORES)

    dram = {}

    def din(name, shape, dt=BF16):
        dram[name] = nc.dram_tensor(name, shape, dt, kind="ExternalInput")
        return dram[name]

    xT = din("xT", [D, L], FP8)
    xR = din("xR", [128, 8, LBLK], BF16)   # residual slice of x
    kgT = din("kgT", [D, E], FP8)
    WqT = din("WqT", [D, D], FP8)
    WkT = din("WkT", [D, D], FP8)
    WkkT = din("WkkT", [D, D], FP8)
    WvT = din("WvT", [D, D], FP8)
    WkvT = din("WkvT", [D, D], FP8)
    WoT = din("WoT", [D, D], FP8)
    WgT = din("WgT", [D, D], FP8)
    bq = din("bq", [128, 8], F32)      # col g = (bias*0.125)[g*128:(g+1)*128]
    bk = din("bk", [128, 8], F32)
    bkk = din("bkk", [128, 8], F32)
    bo = din("bo", [128, 8], F32)
    bge = din("bge", [128, 8], F32)    # bg + Wg[:,1024:] @ kg_mean(batch)
    bvb = din("bvb", [128, D], F32)    # np.tile(bv, (128,1))
    bkvb = din("bkvb", [128, D], F32)

    OUTT = nc.dram_tensor("OUTT", [D, LBLK], F32, kind="ExternalOutput")

    def w8(pool, W, g, tag="w8"):
        t = pool.tile([128, 8, 128], FP8, tag=tag, name=f"{tag}_{W.name}_{g}")
        nc.sync.dma_start(
            t[:], W.ap().rearrange("(kk p) d -> p kk d", p=128)
            [:, :, g * 128:(g + 1) * 128])
        return t

    from contextlib import ExitStack

    with tile.TileContext(nc) as tc, ExitStack() as ctx:
        persist = ctx.enter_context(tc.tile_pool(name="persist", bufs=1))
        wpool = ctx.enter_context(tc.tile_pool(name="wpool", bufs=2))
        spool = ctx.enter_context(tc.tile_pool(name="spool", bufs=2))
        epool = ctx.enter_context(tc.tile_pool(name="epool", bufs=3))
        blkpool = ctx.enter_context(tc.tile_pool(name="blkpool", bufs=2))
        # PSUM: scores 2x[128,1024] (4 banks) + av 1x[65,1024] (2 banks)
        #       + proj 2x[128,512] (2 banks) = 8 banks
        psS = ctx.enter_context(tc.tile_pool(name="psS", bufs=2, space="PSUM"))
        psAV = ctx.enter_context(tc.tile_pool(name="psAV", bufs=1, space="PSUM"))
        psP = ctx.enter_context(tc.tile_pool(name="psP", bufs=2, space="PSUM"))

        # ---- resident loads ----
        xts = persist.tile([128, 8, L], FP8, tag="xts")
        nc.sync.dma_start(xts[:], xT.ap().rearrange("(kk p) l -> p kk l", p=128))
        xrs = persist.tile([128, 8, LBLK], BF16, tag="xrs")
        nc.sync.dma_start(xrs[:], xR.ap())
        kgts = persist.tile([128, 8, E], FP8, tag="kgts")
        nc.sync.dma_start(kgts[:], kgT.ap().rearrange("(kk p) e -> p kk e", p=128))
        biases = {}
        for nm in ("bq", "bk", "bkk", "bo", "bge"):
            t = persist.tile([128, 8], F32, tag=nm, name=nm + "_sb")
            nc.sync.dma_start(t[:], dram[nm].ap())
            biases[nm] = t
        bvbs = persist.tile([128, D], F32, tag="bvbs")
        nc.sync.dma_start(bvbs[:], bvb.ap())
        bkvbs = persist.tile([128, D], F32, tag="bkvbs")
        nc.sync.dma_start(bkvbs[:], bkvb.ap())
        onesv = persist.tile([128, NMC, 4, 1], FP8, tag="onesv")
        nc.vector.memset(onesv[:], WSCALE)
        ones1 = persist.tile([1, 64], BF16, tag="ones1")
        nc.vector.memset(ones1[:], 1.0)

        qts2 = persist.tile([128, 8, LBLK], BF16, tag="qts2")
        outTs = persist.tile([128, 8, LBLK], FP8, tag="outTs")

        # repeat>1 builds a NEFF that runs the whole body `repeat` times --
        # used only by the timing harness (slope removes the RPC overhead).
        for _rep in range(repeat):
         for blk in range(4):
            g0 = 2 * blk
            # kta2: pair-stacked k for this block's 2 pairs
            kta2 = blkpool.tile([128, 2, M], BF16, tag="kta2")
            vslab = blkpool.tile([128, NMC, 4 * VW], FP8, tag="vslab")
            # ones columns for the whole slab in one strided copy
            nc.vector.tensor_copy(
                vslab[:].rearrange("p mc (h c) -> p mc h c", c=VW)[:, :, :, 64:65],
                onesv[:])

            # ---- q projection (heads 4blk..4blk+3 = pairs g0, g0+1) ----
            for g in (g0, g0 + 1) if do_proj else ():
                wq = w8(wpool, WqT, g)
                ps = psP.tile([128, LBLK], F32, tag="psP", name=f"qp{g}")
                for t in range(4):
                    nc.tensor.matmul(ps[:], wq[:, 2 * t:2 * t + 2, :],
                                     xts[:, 2 * t:2 * t + 2, 0:LBLK],
                                     start=(t == 0), stop=(t == 3),
                                     perf_mode=DR)
                nc.vector.tensor_scalar(
                    qts2[:, g, :], ps[:], 0.125 * RSCALE,
                    biases["bq"][:, g:g + 1], OP.mult, OP.add)

            # ---- k projection (full width [128,512] ops into kta2) ----
            for gi, g in enumerate((g0, g0 + 1) if do_proj else ()):
                wk = w8(wpool, WkT, g)
                for lc0 in (0, 2):
                    psa = psP.tile([128, 512], F32, tag="psP",
                                   name=f"kp{g}_{lc0}")
                    psb = psP.tile([128, 512], F32, tag="psP",
                                   name=f"kp{g}_{lc0 + 1}")
                    # t-outer: each wk[t] weight load serves two matmuls
                    for t in range(4):
                        for lc, ps in ((lc0, psa), (lc0 + 1, psb)):
                            nc.tensor.matmul(
                                ps[:], wk[:, 2 * t:2 * t + 2, :],
                                xts[:, 2 * t:2 * t + 2,
                                    lc * 512:(lc + 1) * 512],
                                start=(t == 0), stop=(t == 3), perf_mode=DR)
                    for lc, ps in ((lc0, psa), (lc0 + 1, psb)):
                        nc.vector.tensor_scalar(
                            kta2[:, gi, lc * 512:(lc + 1) * 512], ps[:],
                            RSCALE, biases["bk"][:, g:g + 1],
                            OP.mult, OP.add)
                wkk = w8(wpool, WkkT, g)
                ps = psP.tile([128, E], F32, tag="psP", name=f"kkp{g}")
                for t in range(4):
                    nc.tensor.matmul(ps[:], wkk[:, 2 * t:2 * t + 2, :],
                                     kgts[:, 2 * t:2 * t + 2, :],
                                     start=(t == 0), stop=(t == 3),
                                     perf_mode=DR)
                nc.vector.tensor_scalar(
                    kta2[:, gi, L:M], ps[:], RSCALE,
                    biases["bkk"][:, g:g + 1], OP.mult, OP.add)

            # ---- v projection (dv columns [256*blk, 256*blk+256)) ----
            dlo = 256 * blk
            wv = wpool.tile([128, 8, 256], FP8, tag="wv", name=f"wv{blk}")
            nc.gpsimd.dma_start(
                wv[:], WvT.ap().rearrange("(kk p) d -> p kk d", p=128)
                [:, :, dlo:dlo + 256])
            wkv = wpool.tile([128, 8, 256], FP8, tag="wv", name=f"wkv{blk}")
            nc.gpsimd.dma_start(
                wkv[:], WkvT.ap().rearrange("(kk p) d -> p kk d", p=128)
                [:, :, dlo:dlo + 256])
            for mc in range(NMC if do_proj else 0):
                ps = psP.tile([128, 256], F32, tag="psP", name=f"vp{blk}_{mc}")
                for t in range(4):
                    if mc < 16:
                        lhsT = xts[:, 2 * t:2 * t + 2, mc * 128:(mc + 1) * 128]
                    else:
                        lhsT = kgts[:, 2 * t:2 * t + 2,
                                    (mc - 16) * 128:(mc - 15) * 128]
                    nc.tensor.matmul(ps[:], lhsT,
                                     (wv if mc < 16 else wkv)[:, 2 * t:2 * t + 2, :],
                                     start=(t == 0), stop=(t == 3),
                                     perf_mode=DR)
                bb = bvbs if mc < 16 else bkvbs
                nc.vector.tensor_add(
                    vslab[:, mc, :].rearrange("p (h c) -> p h c", c=VW)
                    [:, :, 0:64],
                    ps[:].rearrange("p (h c) -> p h c", c=64),
                    bb[:, dlo:dlo + 256].rearrange("p (h c) -> p h c", c=64))

            # ---- attention for pairs g0, g0+1 (heads 4blk..4blk+3) ----
            for pr in range(2 if do_attn else 0):
                g = g0 + pr            # pair index == qts2 column
                hh = 2 * pr            # head-in-block of the even head
                avp = psAV.tile([65, 1024], F32, tag="psAV", name=f"av{g}")
                for mc in range(NMC):
                    sp = psS.tile([128, 1024], F32, tag="psS",
                                  name=f"sp{g}_{mc}")
                    # two concurrent K=64 matmuls in different PE row groups
                    nc.tensor.matmul(sp[:, 0:512],
                                     kta2[0:64, pr, mc * 128:(mc + 1) * 128],
                                     qts2[0:64, g, :], start=True, stop=True)
                    nc.tensor.matmul(sp[:, 512:1024],
                                     kta2[64:128, pr, mc * 128:(mc + 1) * 128],
                                     qts2[64:128, g, :], start=True, stop=True)
                    if mc % 2 == 0:
                        et2 = epool.tile([128, 2, 1024], FP8, tag="et",
                                         name=f"et{g}_{mc}")
                    nc.scalar.activation(et2[:, mc % 2, :], sp[:], AF.Exp)
                    if mc % 2 == 1:
                        # DoubleRow fp8 av over the two chunks of et2
                        nc.tensor.matmul(
                            avp[:, 0:512],
                            vslab[:, mc - 1:mc + 1, hh * VW:hh * VW + 65],
                            et2[:, :, 0:512],
                            start=(mc == 1), stop=(mc == NMC - 1),
                            perf_mode=DR)
                        nc.tensor.matmul(
                            avp[:, 512:1024],
                            vslab[:, mc - 1:mc + 1,
                                  (hh + 1) * VW:(hh + 1) * VW + 65],
                            et2[:, :, 512:1024],
                            start=(mc == 1), stop=(mc == NMC - 1),
                            perf_mode=DR)
                # post-process: reciprocal of denominators, broadcast via PE,
                # normalize on DVE into outTs
                # reciprocal_approx_fast reads garbage from PSUM on HW
                # (sim divergence) -- stage the denominator row in SBUF.
                den = spool.tile([1, 1024], F32, tag="den", name=f"den{g}")
                nc.vector.tensor_copy(den[:], avp[64:65, :])
                rec = spool.tile([1, 1024], F32, tag="rec", name=f"rec{g}")
                nc.vector.reciprocal_approx_fast(rec[:], den[:])
                recr = spool.tile([1, 1024], BF16, tag="recr", name=f"recr{g}")
                nc.vector.tensor_copy(recr[:], rec[:])
                avs = spool.tile([64, 1024], F32, tag="avs", name=f"avs{g}")
                nc.vector.tensor_copy(avs[:], avp[0:64, :])
                rpE = psP.tile([64, 512], F32, tag="psP", name=f"rpE{g}")
                nc.tensor.matmul(rpE[:], ones1[:], recr[:, 0:512],
                                 start=True, stop=True)
                rpO = psP.tile([64, 512], F32, tag="psP", name=f"rpO{g}")
                nc.tensor.matmul(rpO[:], ones1[:], recr[:, 512:1024],
                                 start=True, stop=True)
                nc.vector.tensor_mul(outTs[0:64, g, :], avs[:, 0:512], rpE[:])
                nc.vector.tensor_mul(outTs[64:128, g, :], avs[:, 512:1024],
                                     rpO[:])

         # ---- out-proj + gate + residual ----
         for g in range(8 if do_out else 0):
            wo = w8(wpool, WoT, g)
            wg = w8(wpool, WgT, g)
            pp = psP.tile([128, LBLK], F32, tag="psP", name=f"pp{g}")
            for t in range(4):
                nc.tensor.matmul(pp[:], wo[:, 2 * t:2 * t + 2, :],
                                 outTs[:, 2 * t:2 * t + 2, :],
                                 start=(t == 0), stop=(t == 3), perf_mode=DR)
            pj = spool.tile([128, LBLK], F32, tag="pj", name=f"pj{g}")
            nc.vector.tensor_scalar(pj[:], pp[:], RSCALE,
                                    biases["bo"][:, g:g + 1], OP.mult, OP.add)
            gp = psP.tile([128, LBLK], F32, tag="psP", name=f"gp{g}")
            for t in range(4):
                nc.tensor.matmul(gp[:], wg[:, 2 * t:2 * t + 2, :],
                                 outTs[:, 2 * t:2 * t + 2, :],
                                 start=(t == 0), stop=(t == 3), perf_mode=DR)
            gt = spool.tile([128, LBLK], F32, tag="gt", name=f"gt{g}")
            nc.scalar.activation(gt[:], gp[:], AF.Sigmoid,
                                 bias=biases["bge"][:, g:g + 1], scale=RSCALE)
            d1 = spool.tile([128, LBLK], F32, tag="fin", name=f"d1{g}")
            nc.vector.tensor_sub(d1[:], pj[:], xrs[:, g, :])
            d2 = spool.tile([128, LBLK], F32, tag="fin", name=f"d2{g}")
            nc.vector.tensor_mul(d2[:], d1[:], gt[:])
            fo = spool.tile([128, LBLK], F32, tag="fin", name=f"fo{g}")
            nc.vector.tensor_add(fo[:], d2[:], xrs[:, g, :])
            nc.sync.dma_start(OUTT.ap()[g * 128:(g + 1) * 128, :], fo[:])

    nc.compile()
    return nc


def kernel(x, kg_embeds, Wq, bq, Wk, bk, Wv, bv, Wkk, bkk, Wkv, bkv,
           Wo, bo, Wg, bg):
    import ml_dtypes
    bf16 = ml_dtypes.bfloat16
    f8 = ml_dtypes.float8_e4m3

    x = np.asarray(x, np.float32)
    kg_embeds = np.asarray(kg_embeds, np.float32)
    ws = {k: np.asarray(v, np.float32) for k, v in dict(
        Wq=Wq, bq=bq, Wk=Wk, bk=bk, Wv=Wv, bv=bv, Wkk=Wkk, bkk=bkk,
        Wkv=Wkv, bkv=bkv, Wo=Wo, bo=bo, Wg=Wg, bg=bg).items()}

    if "nc" not in _CACHE:
        _CACHE["nc"] = _build()
    nc = _CACHE["nc"]

    def col8(v):
        return np.ascontiguousarray(v.reshape(8, 128).T)

    def w16(w):
        # weights pre-scaled by WSCALE=16 before fp8 so they stay normal
        return np.ascontiguousarray((w.T * 16.0).astype(f8))

    # pair-stacked bias layouts: column g holds dims for head pair g
    # (even head dims at rows 0:64, odd head dims at rows 64:128) -- which
    # is exactly the contiguous [g*128:(g+1)*128] slice, same as col8.
    shared = {
        "WqT": w16(ws["Wq"]),
        "WkT": w16(ws["Wk"]),
        "WkkT": w16(ws["Wkk"]),
        "WvT": w16(ws["Wv"]),
        "WkvT": w16(ws["Wkv"]),
        "WoT": w16(ws["Wo"]),
        "WgT": w16(ws["Wg"][:, :D]),
        "bq": col8(ws["bq"] * 0.125),
        "bk": col8(ws["bk"]),
        "bkk": col8(ws["bkk"]),
        "bo": col8(ws["bo"]),
        # vslab holds 16*v (weights pre-scaled, bias scaled to match); the
        # 16 cancels against the 16.0 ones-column in the softmax ratio
        "bvb": np.ascontiguousarray(np.tile(ws["bv"] * 16.0, (128, 1))),
        "bkvb": np.ascontiguousarray(np.tile(ws["bkv"] * 16.0, (128, 1))),
    }

    in_maps = []
    for c in range(N_CORES):
        b, j = divmod(c, 4)
        # roll the core's query block to columns [0, 512); k/v attend over
        # all columns, so their (rolled) order is irrelevant to softmax
        xb = np.ascontiguousarray(np.roll(x[b].T, -j * LBLK, axis=1))
        kgm = kg_embeds[b].mean(axis=0)
        bge = ws["bg"] + ws["Wg"][:, D:] @ kgm
        m = dict(shared)
        m["xT"] = xb.astype(f8)
        m["xR"] = np.ascontiguousarray(
            xb[:, :LBLK].reshape(8, 128, LBLK).transpose(1, 0, 2)).astype(bf16)
        m["kgT"] = np.ascontiguousarray(kg_embeds[b].T.astype(f8))
        m["bge"] = col8(bge)
        in_maps.append(m)

    _CACHE["in_maps"] = in_maps
    res = run_bass_kernel_spmd(nc, in_maps, core_ids=list(range(N_CORES)))
    out = np.empty((B, L, D), np.float32)
    for c in range(N_CORES):
        b, j = divmod(c, 4)
        out[b, j * LBLK:(j + 1) * LBLK, :] = res.results[c]["OUTT"].T
    return out
